# revision 23
# baseline (speedup 1.0000x reference)
"""CASSI shear kernel for Trainium2 (Bass/Tile), 8-core SPMD.

Computes, for full inputs x (1, 1024, 1024, 31) and ca (1, 1024, 1024, 1):
    y1[m, n, l] = x[m, n, l] * ca[m, n]
    out[m, j]   = sum_{n+l=j} y1[m, n, l]       (j in [0, 1054))
returning (1, 1024, 1054, 1) float32.

Sharding: rows m across 8 cores (128 rows/core = one full SBUF partition
block); no cross-core communication.

Per-core structure (DMA-bound; ~17MB of HBM traffic at ~350GB/s):
  - x streams in n-chunks (sync/SP HWDGE ring); ca + ident + out stores
    ride the scalar/ACT ring so they never delay the x stream.
  - Vector engine: y = x * ca (broadcast over l), written bf16 into ONE
    contiguous SBUF tile covering the whole (n, l) plane, chunk by chunk.
  - Tensor engine: shear scatter-add as identity-weight bf16 matmuls
    into PSUM, one piece per (target-column window, l).  Windows are
    PSUM-bank aligned; piece dst is a contiguous column range (g=1 -
    overlapping multi-l dst APs measured ~5x slower).  PSUM zero-init is
    implicit: the first matmul per bank uses start=True (bank-wide
    has_written clear), later matmuls overwrite-or-accumulate per
    element, so no explicit zeroing matmuls are needed.
  - Scalar engine evacuates each PSUM bank to SBUF as soon as its last
    matmul retires; finished column ranges stream out mid-kernel.
"""

import sys

import numpy as np

if "/opt/trn_rl_repo" not in sys.path:
    sys.path.insert(0, "/opt/trn_rl_repo")

M, N, L = 1024, 1024, 31
ONC = N + L - 1  # 1054
NCORES = 8
R = M // NCORES  # 128 rows per core
BANK = 512  # PSUM bank size in fp32 elements

# n-chunk sizes for the DMA/multiply pipeline (sum == N).  Tapered at the
# end so the serial tail (last multiply -> last shear pieces -> evacuate
# -> store) is short.
CHUNKS = [64, 64, 128, 128, 128, 128, 128, 128, 64, 32, 32]
# Target-column windows for the shear matmuls.  Each window must lie
# inside one PSUM bank; finer windows at the start get the PE going
# ~13us sooner, finer windows at the end shorten the tail.
WINDOWS = [(0, 64), (64, 128), (128, 256), (256, 512), (512, 768),
           (768, 960), (960, 992), (992, 1024), (1024, ONC)]

# The (n,l) -> l-major reorder is inescapably strided on one side (x is
# n-major in HBM), and every engine processes non-contiguous access
# patterns at ~0.5 elem/cycle.  Measured rates per element:
#   DVE  l-major multiply (strided x read):   ~1.81 ns
#   DVE  n-major multiply (contiguous runs):  ~1.04 ns
#   PE   matmul, contiguous rhs (l-major y):  ~0.45 ns
#   PE   matmul, strided rhs (n-major y):     ~2.0  ns  (clock-insensitive)
# Splitting l-space balances DVE and PE at ~43us each, under the ~47us
# DMA floor: l < LSPLIT goes l-major (DVE pays the stride, PE fast),
# l >= LSPLIT stays n-major (DVE fast, PE pays the stride).
LSPLIT = 14
NL2 = L - LSPLIT

# PSUM accumulator split: [0,512) and [512,992) evacuate + store while the
# tail matmuls (target cols >= 992, which depend on the last chunk) still
# run; only 62 columns remain to move at the very end.  Each range must fit
# a 512-col PSUM bank (tiles are padded to keep bank alignment).
BANK_BOUNDS = [(0, 512), (512, 992), (992, ONC)]

_cached_nc = {}


def _pieces():
    """Shear matmuls as (bank, l, n0, w, start, stop).

    Piece (window [ta,tb), l) reads y1[:, n, l] for n in [n0, n0+w) and
    accumulates into psum bank columns [n0+l, n0+w+l) (all inside bank
    ta//BANK).  start/stop mark the first/last matmul per bank.  Within a
    window the strided (l >= LSPLIT) pieces go first: their rate does not
    depend on the PE clock, so they double as HAM warm-up for the fast
    contiguous pieces that follow."""
    out = []
    for ta, tb in WINDOWS:
        b = next(
            i for i, (ba, bz) in enumerate(BANK_BOUNDS)
            if ba <= ta and tb <= bz
        )
        win = []
        for l in range(L):
            n0 = max(ta - l, 0)
            n1 = min(tb - l, N)
            w = n1 - n0
            if w <= 0:
                continue
            win.append([b, l, n0, w, False, False])
        win.sort(key=lambda p: p[1] < LSPLIT)  # strided first
        out.extend(win)
    first, last = {}, {}
    for i, p in enumerate(out):
        first.setdefault(p[0], i)
        last[p[0]] = i
    for i in first.values():
        out[i][4] = True
    for i in last.values():
        out[i][5] = True
    return out


def _build_nc(loop_iters=None, variant="full"):
    key = (loop_iters, variant)
    if key in _cached_nc:
        return _cached_nc[key]

    import concourse.bass as bass
    import concourse.mybir as mybir
    from concourse import bacc
    from concourse.tile import TileContext

    f32 = mybir.dt.float32
    bf16 = mybir.dt.bfloat16
    nc = bacc.Bacc("TRN2")

    xin = nc.dram_tensor("x", (R, N * L), f32, kind="ExternalInput")
    cain = nc.dram_tensor("ca", (R, N), f32, kind="ExternalInput")
    identin = nc.dram_tensor("ident", (R, R), bf16, kind="ExternalInput")
    outd = nc.dram_tensor("out", (R, ONC), f32, kind="ExternalOutput")

    maxchunk = max(CHUNKS)
    pieces = _pieces()
    # evacuate bank b after piece index evac_after[b] (its stop piece)
    evac_after = {}
    for i, (b, _, _, _, _, stop) in enumerate(pieces):
        if stop:
            evac_after[i] = b

    with TileContext(nc) as tc:
        with (
            tc.tile_pool(name="cp", bufs=1) as cp,
            tc.tile_pool(name="xp", bufs=3) as xp,
            tc.tile_pool(name="yp", bufs=1) as yp,
            tc.tile_pool(name="pp", bufs=1, space="PSUM") as pp,
        ):
            ca_t = cp.tile([R, N], f32)
            idb = cp.tile([R, R], bf16, tag="idb")
            acc = cp.tile([R, ONC], f32, tag="acc")
            # y1 for l < LSPLIT, l-major: ylm[p, l*N + n]
            ylm = yp.tile([R, N * LSPLIT], bf16, tag="ylm")
            # y1 for l >= LSPLIT, n-major: ynm[p, n*NL2 + (l-LSPLIT)]
            ynm = yp.tile([R, N * NL2], bf16, tag="ynm")
            # tiles padded to 512 so each starts on a PSUM bank boundary
            bank0 = pp.tile([R, BANK], f32, tag="b0")
            bank1 = pp.tile([R, BANK], f32, tag="b1")
            bank2 = pp.tile([R, ONC - 992], f32, tag="b2")
            banks = [bank0, bank1, bank2]

            # ident on the ACT HWDGE ring; ca on the sync ring AHEAD of the
            # x chunks — the ACT-ring queue drains ~5x slower while the x
            # stream hogs the SDMA round-robin, which would stall the first
            # multiply.  ca loads in two parts so chunk 0's multiply only
            # waits for the small head.
            nc.scalar.dma_start(out=idb[:], in_=identin[:])
            nc.sync.dma_start(out=ca_t[:, 0:256], in_=cain[:, 0:256])

            def body():
                ylmv = ylm[:]
                ylmpart = [int(ylmv.ap[0][0]), int(ylmv.ap[0][1])]
                ynmv = ynm[:]
                ynmpart = [int(ynmv.ap[0][0]), int(ynmv.ap[0][1])]
                cav = ca_t[:]
                capart = [int(cav.ap[0][0]), int(cav.ap[0][1])]
                n0 = 0
                for ci, cn in enumerate(CHUNKS):
                    xt = xp.tile([R, maxchunk * L], f32, tag="xchunk")
                    nc.sync.dma_start(
                        out=xt[:, 0 : cn * L],
                        in_=xin[:, n0 * L : (n0 + cn) * L],
                    )
                    if ci == 0:
                        # rest of ca right behind chunk 0 on the sync ring
                        nc.sync.dma_start(
                            out=ca_t[:, 256:N], in_=cain[:, 256:N]
                        )
                    xv = xt[:, 0 : cn * L]
                    xpart = [int(xv.ap[0][0]), int(xv.ap[0][1])]
                    # l-major half: (l outer, n inner) - contiguous writes,
                    # x reads strided by L
                    ydst = bass.AP(
                        ylmv.tensor, ylmv.offset + n0,
                        [ylmpart, [N, LSPLIT], [1, cn]],
                    )
                    xsrc = bass.AP(
                        xv.tensor, xv.offset,
                        [xpart, [1, LSPLIT], [L, cn]],
                    )
                    casrc = bass.AP(
                        cav.tensor, cav.offset + n0,
                        [capart, [0, LSPLIT], [1, cn]],
                    )
                    nc.vector.tensor_tensor(
                        ydst, xsrc, casrc, mybir.AluOpType.mult
                    )
                    # n-major half: (n outer, l inner) - contiguous runs on
                    # both sides
                    ydst = bass.AP(
                        ynmv.tensor, ynmv.offset + n0 * NL2,
                        [ynmpart, [NL2, cn], [1, NL2]],
                    )
                    xsrc = bass.AP(
                        xv.tensor, xv.offset + LSPLIT,
                        [xpart, [L, cn], [1, NL2]],
                    )
                    casrc = bass.AP(
                        cav.tensor, cav.offset + n0,
                        [capart, [1, cn], [0, NL2]],
                    )
                    nc.vector.tensor_tensor(
                        ydst, xsrc, casrc, mybir.AluOpType.mult
                    )
                    n0 += cn

                for i, (b, l, pn0, w, start, stop) in enumerate(pieces):
                    if l < LSPLIT:
                        rhs = bass.AP(
                            ylmv.tensor, ylmv.offset + l * N + pn0,
                            [ylmpart, [1, w]],
                        )
                    else:
                        rhs = bass.AP(
                            ynmv.tensor,
                            ynmv.offset + pn0 * NL2 + (l - LSPLIT),
                            [ynmpart, [NL2, w]],
                        )
                    c0 = pn0 + l - BANK_BOUNDS[b][0]
                    dst = banks[b][:, c0 : c0 + w]
                    nc.tensor.matmul(
                        dst, idb[:], rhs,
                        start=start, stop=stop, skip_group_check=True,
                    )
                    eb = evac_after.get(i)
                    if eb is None:
                        continue
                    # Evacuate the finished bank on the (idle) scalar engine
                    # and stream the columns out.
                    a, z = BANK_BOUNDS[eb]
                    nc.scalar.copy(acc[:, a:z], banks[eb][:, 0 : z - a])
                    # bank 0 finishes mid-kernel while sync still streams x:
                    # store it from the ACT ring.  Banks 1/2 finish late when
                    # sync is idle: store from the sync ring so stores don't
                    # queue behind ACT work.
                    eng = nc.scalar if eb == 0 else nc.sync
                    eng.dma_start(out=outd[:, a:z], in_=acc[:, a:z])

            if loop_iters is None:
                body()
            else:
                with tc.For_i(0, loop_iters, 1):
                    body()

    nc.finalize()
    _cached_nc[key] = nc
    return nc


_IDENT = None


def _ident_bf16():
    global _IDENT
    if _IDENT is None:
        try:
            from ml_dtypes import bfloat16
        except ImportError:
            from jax.numpy import bfloat16
        _IDENT = np.eye(R, dtype=bfloat16)
    return _IDENT


def _run(x_slab, ca_slab, loop_iters=None, variant="full", **run_kwargs):
    """x_slab (M, N*L) f32, ca_slab (M, N) f32 -> ((M, ONC) f32, results)."""
    from concourse.bass_utils import run_bass_kernel_spmd

    nc = _build_nc(loop_iters, variant)
    ident = _ident_bf16()
    in_maps = []
    for c in range(NCORES):
        in_maps.append(
            {
                "x": np.ascontiguousarray(x_slab[c * R : (c + 1) * R]),
                "ca": np.ascontiguousarray(ca_slab[c * R : (c + 1) * R]),
                "ident": ident,
            }
        )
    res = run_bass_kernel_spmd(nc, in_maps, core_ids=list(range(NCORES)), **run_kwargs)
    out = np.concatenate(
        [np.asarray(res.results[c]["out"]) for c in range(NCORES)], axis=0
    )
    return out, res


def kernel(x, ca):
    x = np.ascontiguousarray(np.asarray(x, dtype=np.float32).reshape(M, N * L))
    ca = np.ascontiguousarray(np.asarray(ca, dtype=np.float32).reshape(M, N))
    out, _ = _run(x, ca)
    return out.reshape(1, M, ONC, 1)
